# revision 12
# baseline (speedup 1.0000x reference)
"""Trainium2 Bass kernel for nn_CrossAttentionLayer (B=8, N=2048, Q=256, D=1024, H=16).

Data-parallel over batch (1 sample per NeuronCore, 8 cores).

Device strategy (per core):
  - All GEMMs except scores run as fp8e4 DoubleRow matmuls (two 128-deep
    contraction planes per instruction, 0.5 cycles/row = 2x bf16). Dual-fp8
    LDWEIGHTS requires per-plane free dim in {32,64,128}.
  - Scores are bf16 "pair-packed": heads (2m, 2m+1) share one matmul with a
    block-diagonal rhs (qt halves zero-padded), giving full 128-partition
    contraction and 512-wide streams.
  - attn@V packs both heads of a pair in one [128,2,128] lhsT; the output's
    diagonal quadrants are the two heads' contributions, off-diagonal
    quadrants are ignored (free: matmul cost scales with the moving dim).
    Softmax denominators come from an all-ones [128,2,64] lhsT against the
    same exp tiles - pre-broadcast across 64 partitions, so normalization is
    one reciprocal + two multiplies on the DVE, no PE broadcast.
  - The main loop interleaves, per head-pair i: K-proj(i) chunks, scores(i)
    chunks, attn@V(i-2) - so the PE queue never drains (keeps the 2.4GHz
    p-state) while the ACT engine pipelines exp 2 pairs behind.

Host-side preprocessing:
  - weights scaled x16 before fp8 cast (keeps values in e4m3's resolved
    range); compensating 1/2048 folded into the qt eviction, 1/256 into the
    out-proj eviction; exp computed as exp(score - 2) (softmax-invariant).
  - V bias folded through the output projection; K bias dropped (softmax
    invariant); resid = queries + b_out + w_out @ b_v added at the end.
"""

import numpy as np
import ml_dtypes
from contextlib import ExitStack

import concourse.bass as bass
import concourse.mybir as mybir
import concourse.tile as tile
from concourse import bacc
from concourse.bass_utils import run_bass_kernel_spmd

F32 = mybir.dt.float32
BF16 = mybir.dt.bfloat16
F8 = mybir.dt.float8e4
AF = mybir.ActivationFunctionType
DR = mybir.MatmulPerfMode.DoubleRow

B, N, Q, D, H = 8, 2048, 256, 1024, 16
N_CORES = 8
W_SCALE = 16.0
EXP_BIAS = -2.0


def build(N=N, Q=Q, D=D, H=H):
    HD = D // H            # 64
    KT = D // 256          # 4 DoubleRow contraction steps
    NT = N // 128          # 16 source-token tiles
    MT = D // 128          # 8 pairs (2 heads of 64 dims per 128-row tile)
    QT = Q // 128          # 2
    assert Q == 256 and HD == 64

    nc = bacc.Bacc(None, target_bir_lowering=False)
    src8 = nc.declare_dram_parameter("src8", [128, KT, 2, N], F8, isOutput=False)
    qry8 = nc.declare_dram_parameter("qry8", [128, KT, 2, Q], F8, isOutput=False)
    wv8 = nc.declare_dram_parameter("wv8", [128, KT, 2, D], F8, isOutput=False)
    wk8 = nc.declare_dram_parameter("wk8", [128, KT, 2, D], F8, isOutput=False)
    wq8 = nc.declare_dram_parameter("wq8", [128, KT, 2, D], F8, isOutput=False)
    wo8 = nc.declare_dram_parameter("wo8", [128, KT, 2, D], F8, isOutput=False)
    bq16 = nc.declare_dram_parameter("bq16", [128, MT], F32, isOutput=False)
    resid = nc.declare_dram_parameter("resid", [128, QT, D], F32, isOutput=False)
    out = nc.declare_dram_parameter("out", [Q, D], F32, isOutput=True)

    with tile.TileContext(nc) as tc, ExitStack() as ctx:
        ps_pool = ctx.enter_context(tc.tile_pool(name="psA", bufs=2, space="PSUM"))
        ss_pool = ctx.enter_context(tc.tile_pool(name="psS", bufs=2, space="PSUM"))
        po_pool = ctx.enter_context(tc.tile_pool(name="psO", bufs=1, space="PSUM"))
        dn_pool = ctx.enter_context(tc.tile_pool(name="psD", bufs=1, space="PSUM"))
        res_pool = ctx.enter_context(tc.tile_pool(name="res", bufs=1))
        exp_pool = ctx.enter_context(tc.tile_pool(name="expp", bufs=3))
        nrm_pool = ctx.enter_context(tc.tile_pool(name="nrm", bufs=2))
        osb_pool = ctx.enter_context(tc.tile_pool(name="osb", bufs=2))

        # ---- resident SBUF tensors ----
        src_sb = res_pool.tile([128, KT, 2, N], F8, tag="src")
        qry_sb = res_pool.tile([128, KT, 2, Q], F8, tag="qry")
        wq_sb = res_pool.tile([128, KT, 2, D], F8, tag="wq")
        wk_sb = res_pool.tile([128, KT, 2, D], F8, tag="wk")
        wv_sb = res_pool.tile([128, KT, 2, D], F8, tag="wv")
        wo_sb = res_pool.tile([128, KT, 2, D], F8, tag="wo")
        kt_sb = res_pool.tile([128, MT, N], BF16, tag="kt")
        v2_sb = res_pool.tile([128, NT, MT, 128], F8, tag="v2")
        qt2_sb = res_pool.tile([128, MT, 2 * Q], BF16, tag="qt2")
        ao_sb = res_pool.tile([128, MT, Q], F8, tag="ao")
        bq_sb = res_pool.tile([128, MT], F32, tag="bq")
        res_sb = res_pool.tile([128, QT, D], F32, tag="res")
        ones8_sb = res_pool.tile([128, 2, HD], F8, tag="ones8")
        ebias_sb = res_pool.tile([128, 1], F32, tag="ebias")

        # ---- DMA spread across engine queues so compute starts early:
        # scalar: wq (first need) + wo (last need); vector: wv; gpsimd: wk+bq;
        # sync: qry, src (4 n-chunks so V proj starts after the first), resid.
        nc.scalar.dma_start(out=wq_sb[:, :, :, 0:512], in_=wq8[:, :, :, 0:512])
        nc.scalar.dma_start(out=wq_sb[:, :, :, 512:D], in_=wq8[:, :, :, 512:D])
        nc.scalar.dma_start(out=wv_sb, in_=wv8[:, :, :, :])
        nc.gpsimd.dma_start(out=bq_sb, in_=bq16[:, :])
        nc.gpsimd.dma_start(out=wk_sb, in_=wk8[:, :, :, :])
        nc.sync.dma_start(out=qry_sb, in_=qry8[:, :, :, :])
        for h_ in range(2):
            nc.sync.dma_start(
                out=src_sb[:, :, :, h_ * (N // 2):(h_ + 1) * (N // 2)],
                in_=src8[:, :, :, h_ * (N // 2):(h_ + 1) * (N // 2)],
            )

        nc.gpsimd.memset(qt2_sb, 0.0)
        nc.vector.memset(ones8_sb, 1.0)
        nc.vector.memset(ebias_sb, EXP_BIAS)

        # ---- P1: Q projection -> qt2 (pair-packed, zero-padded) ----
        for m in range(MT):
            ps = ps_pool.tile([128, 512], F32, tag="ps", name=f"psq{m}")
            for k in range(KT):
                nc.tensor.matmul(
                    ps[:, 0:Q],
                    lhsT=wq_sb[:, k, :, m * 128:(m + 1) * 128],
                    rhs=qry_sb[:, k, :, :],
                    start=(k == 0), stop=(k == KT - 1), perf_mode=DR,
                )
            # qt2 = (16*q + 16*bq)/2048 = (q + bq)/128
            nc.vector.tensor_scalar(
                out=qt2_sb[0:64, m, 0:Q], in0=ps[0:64, 0:Q],
                scalar1=bq_sb[0:64, m:m + 1], scalar2=1.0 / 2048.0,
                op0=mybir.AluOpType.add, op1=mybir.AluOpType.mult,
            )
            nc.vector.tensor_scalar(
                out=qt2_sb[64:128, m, Q:2 * Q], in0=ps[64:128, 0:Q],
                scalar1=bq_sb[64:128, m:m + 1], scalar2=1.0 / 2048.0,
                op0=mybir.AluOpType.add, op1=mybir.AluOpType.mult,
            )

        # ---- P2: V projection -> v2_sb fp8 (16*v), pair-major layout ----
        for t in range(NT):
            pv = [ps_pool.tile([128, 512], F32, tag="ps", name=f"psv{t}_{c}")
                  for c in range(2)]
            for k in range(KT):
                for c in range(2):
                    nc.tensor.matmul(
                        pv[c][:],
                        lhsT=src_sb[:, k, :, t * 128:(t + 1) * 128],
                        rhs=wv_sb[:, k, :, c * 512:(c + 1) * 512],
                        start=(k == 0), stop=(k == KT - 1), perf_mode=DR,
                    )
            for c in range(2):
                nc.vector.tensor_copy(
                    out=v2_sb[:, t, c * 4:(c + 1) * 4, :],
                    in_=pv[c][:].rearrange("p (mp c) -> p mp c", mp=4),
                )

        # late DMAs: wo/res are only needed by the output projection
        nc.scalar.dma_start(out=wo_sb, in_=wo8[:, :, :, :])
        nc.sync.dma_start(out=res_sb, in_=resid[:, :, :])

        # ---- P3: K proj + scores + exp + attn@V, interleaved per pair ----
        expts = {}

        def emit_k_chunkpair(m, cp):
            pk = [ps_pool.tile([128, 512], F32, tag="ps", name=f"psk{m}_{2*cp+j}")
                  for j in range(2)]
            for k in range(KT):
                for j in range(2):
                    nc.tensor.matmul(
                        pk[j][:],
                        lhsT=wk_sb[:, k, :, m * 128:(m + 1) * 128],
                        rhs=src_sb[:, k, :, (2 * cp + j) * 512:(2 * cp + j + 1) * 512],
                        start=(k == 0), stop=(k == KT - 1), perf_mode=DR,
                    )
            for j in range(2):
                nc.vector.tensor_copy(
                    out=kt_sb[:, m, (2 * cp + j) * 512:(2 * cp + j + 1) * 512],
                    in_=pk[j],
                )

        def emit_score_chunk(m, c):
            # chunk c covers n tiles 2c, 2c+1 -> needs K chunk c//2 done
            ss = ss_pool.tile([128, 2, 512], F32, tag="ss", name=f"ss{m}_{c}")
            for j in range(2):
                nt = 2 * c + j
                nc.tensor.matmul(
                    ss[:, j, :],
                    lhsT=kt_sb[:, m, nt * 128:(nt + 1) * 128],
                    rhs=qt2_sb[:, m, :],
                    start=True, stop=True,
                )
            nc.scalar.activation(
                out=expts[m][:, 2 * c:2 * c + 2, :], in_=ss[:],
                func=AF.Exp, bias=ebias_sb[:],
            )

        def emit_attn_po(m, po_t):
            for j in range(NT // 2):
                nc.tensor.matmul(
                    po_t[:],
                    lhsT=v2_sb[:, 2 * j:2 * j + 2, m, :],
                    rhs=expts[m][:, 2 * j:2 * j + 2, :],
                    start=(j == 0), stop=(j == NT // 2 - 1), perf_mode=DR,
                )

        def emit_attn_dn(m, po_t, dn_t):
            for j in range(NT // 2):
                nc.tensor.matmul(
                    dn_t[:],
                    lhsT=ones8_sb[:],
                    rhs=expts[m][:, 2 * j:2 * j + 2, :],
                    start=(j == 0), stop=(j == NT // 2 - 1), perf_mode=DR,
                )
            rcp = nrm_pool.tile([HD, 2 * Q], F32, tag="rcp", name=f"rcp{m}")
            nc.vector.reciprocal_approx_fast(out=rcp, in_=dn_t[:])
            # diagonal quadrants: head 2m rows 0:64 cols 0:Q, head 2m+1
            # rows 64:128 cols Q:2Q (denominators are row-broadcast already)
            nc.vector.tensor_mul(
                ao_sb[0:HD, m, :], po_t[0:HD, 0:Q], rcp[:, 0:Q],
            )
            nc.vector.tensor_mul(
                ao_sb[HD:128, m, :], po_t[HD:128, Q:2 * Q], rcp[:, Q:2 * Q],
            )

        for i in range(MT + 2):
            if i < MT:
                m = i
                expts[m] = exp_pool.tile([128, NT, 512], F8, tag="exp", name=f"expt{m}")
                emit_k_chunkpair(m, 0)
                emit_score_chunk(m, 0)
                emit_score_chunk(m, 1)
                if 0 <= i - 2:
                    po_t = po_pool.tile([128, 512], F32, tag="po", name=f"po{i-2}")
                    emit_attn_po(i - 2, po_t)
                emit_score_chunk(m, 2)
                emit_score_chunk(m, 3)
                if 0 <= i - 2:
                    dn_t = dn_pool.tile([HD, 2 * Q], F32, tag="dn", name=f"dn{i-2}")
                    emit_attn_dn(i - 2, po_t, dn_t)
                    expts.pop(i - 2)
                emit_k_chunkpair(m, 1)
                for c in range(4, 8):
                    emit_score_chunk(m, c)
            else:
                po_t = po_pool.tile([128, 512], F32, tag="po", name=f"po{i-2}")
                emit_attn_po(i - 2, po_t)
                dn_t = dn_pool.tile([HD, 2 * Q], F32, tag="dn", name=f"dn{i-2}")
                emit_attn_dn(i - 2, po_t, dn_t)
                expts.pop(i - 2)

        # ---- P5: output projection + residual ----
        for qt in range(QT):
            for c in range(2):
                ps = ps_pool.tile([128, 512], F32, tag="ps", name=f"psf{qt}_{c}")
                for k in range(KT):
                    nc.tensor.matmul(
                        ps[:],
                        lhsT=ao_sb[:, 2 * k:2 * k + 2, qt * 128:(qt + 1) * 128],
                        rhs=wo_sb[:, k, :, c * 512:(c + 1) * 512],
                        start=(k == 0), stop=(k == KT - 1), perf_mode=DR,
                    )
                osb = osb_pool.tile([128, 512], F32, tag="osb1", name=f"osb1_{qt}_{c}")
                nc.scalar.activation(
                    out=osb, in_=ps[:], func=AF.Copy, scale=1.0 / 256.0
                )
                osb2 = osb_pool.tile([128, 512], F32, tag="osb2", name=f"osb2_{qt}_{c}")
                nc.vector.tensor_add(
                    osb2[:], osb[:], res_sb[:, qt, c * 512:(c + 1) * 512]
                )
                nc.sync.dma_start(
                    out=out[qt * 128:(qt + 1) * 128, c * 512:(c + 1) * 512], in_=osb2
                )

    nc.finalize()
    return nc


_NC_CACHE = {}


def _get_nc():
    key = (N, Q, D, H)
    if key not in _NC_CACHE:
        _NC_CACHE[key] = build()
    return _NC_CACHE[key]


def make_in_maps(sources, queries, w_in, b_in, w_out, b_out):
    FP8 = ml_dtypes.float8_e4m3
    sources = np.asarray(sources, dtype=np.float32)
    queries = np.asarray(queries, dtype=np.float32)
    w_in = np.asarray(w_in, dtype=np.float32)
    b_in = np.asarray(b_in, dtype=np.float32)
    w_out = np.asarray(w_out, dtype=np.float32)
    b_out = np.asarray(b_out, dtype=np.float32)

    w_q, w_k, w_v = w_in[0:D], w_in[D:2 * D], w_in[2 * D:3 * D]
    b_q, b_v = b_in[0:D], b_in[2 * D:3 * D]
    # b_k dropped: constant shift along softmax axis
    def pre(a):
        # [din, X] -> [128, KT, 2, X] with din = kt*256 + two*128 + p
        return np.ascontiguousarray(
            a.reshape(D // 256, 2, 128, -1).transpose(2, 0, 1, 3))

    wq8 = pre((W_SCALE * w_q.T).astype(FP8))
    wk8 = pre((W_SCALE * w_k.T).astype(FP8))
    wv8 = pre((W_SCALE * w_v.T).astype(FP8))
    wo8 = pre((W_SCALE * w_out.T).astype(FP8))
    bq16 = np.ascontiguousarray(
        (W_SCALE * b_q).astype(np.float32).reshape(D // 128, 128).T)
    bout_eff = b_out + w_out @ b_v

    in_maps = []
    for b in range(B):
        in_maps.append({
            "src8": pre(sources[b].T.astype(FP8)),
            "qry8": pre(queries[b].T.astype(FP8)),
            "wv8": wv8, "wk8": wk8, "wq8": wq8, "wo8": wo8,
            "bq16": bq16,
            "resid": np.ascontiguousarray(
                (queries[b] + bout_eff[None, :]).astype(np.float32)
                .reshape(Q // 128, 128, D).transpose(1, 0, 2)),
        })
    return in_maps


def kernel(sources, queries, w_in, b_in, w_out, b_out, _trace=False):
    nc = _get_nc()
    in_maps = make_in_maps(sources, queries, w_in, b_in, w_out, b_out)
    res = run_bass_kernel_spmd(nc, in_maps, core_ids=list(range(N_CORES)), trace=_trace)
    out = np.stack([res.results[b]["out"] for b in range(B)], axis=0)
    if _trace:
        kernel.last_exec_time_ns = res.exec_time_ns
        kernel.last_results = res
    return out
